# revision 27
# baseline (speedup 1.0000x reference)
"""Trainium2 Bass kernel for causal GQA attention — bf16 GEMMs + fp8 full-block
attention.

Same sharding/schedule skeleton as the original compensated-fp8 baseline
(8 cores = 2 batch x 4 head-groups; head-transposed [D, T] layouts, no
on-chip transposes), with dtypes chosen from measured PE rates (mb.py):
  - one fp8 DoubleRow matmul (256 contraction rows, N=512) = 305 ns vs
    bf16's 2x260 = 520 ns, so the old 3-term compensated-fp8 GEMM scheme
    (719 ns) is 1.38x SLOWER than plain bf16 -> all projections + wo run
    plain bf16 (~0.4% error, well inside the 2e-2 gate).
  - attention full (off-diagonal) blocks DO win with fp8: exp writes e4m3
    pairs (logits are O(4), far under e4m3's 240 max), then one DoubleRow
    AV (fp8 v) and one DoubleRow ones-denominator per 2 k-blocks — vs
    2x260 + 2x265 bf16. Diagonal blocks stay bf16 for the causal mask
    multiply. fp8 noise here stays below the bf16 error floor.

Layout: qT/kT = w.T @ x.T from PE; scores S^T[k,q] = kT.T @ qT (bf16);
AV + denominator accumulate in PSUM per (head, 512-wide q-chunk) unit.
Normalize writes bf16 attention-out into aob[d, h, t]; the row-parallel wo
GEMM consumes it per t-block, host sums 4 partials per batch.

Schedule: K projection runs e-block-outer across all 8 (kv-head, chunk)
accumulators so PE consumes x as the per-e-block-interleaved wk/x DMA
delivers it; V then Q run chunk-major; attention chunk-ascending with each
chunk's wo spread one t-block per head one chunk later, so the final
normalize latency is overlapped with wo matmuls.
"""

import numpy as np
import ml_dtypes

BF16 = ml_dtypes.bfloat16
F16 = np.float16

B, T, E = 2, 2048, 2048
H, D = 16, 128
KVH = 8
THETA = 10000.0
P = 128
EB = E // P          # 16 contraction e-blocks
CH = 512             # q-chunk width
NTQ = T // CH        # 4 q chunks
NTB = T // P         # 16 t blocks
NH = H // 4          # 4 q heads per core
NKV = 2              # kv heads per core
SCALE = float(D) ** -0.5

_NC_CACHE = {}


def _build_nc(reps=1):
    import concourse.mybir as mybir
    import concourse.tile as tile
    from concourse import bacc

    nc = bacc.Bacc(None, target_bir_lowering=False)
    dt = mybir.dt
    f32, bf16, f16, f8 = dt.float32, dt.bfloat16, dt.float16, dt.float8e4
    DR = mybir.MatmulPerfMode.DoubleRow
    Exp = mybir.ActivationFunctionType.Exp
    Copy = mybir.ActivationFunctionType.Copy

    xb_d = nc.dram_tensor("xb", [E, T], bf16, kind="ExternalInput")
    wqb_d = nc.dram_tensor("wqb", [E, NH * D], bf16, kind="ExternalInput")
    wkb_d = nc.dram_tensor("wkb", [E, NKV * D], bf16, kind="ExternalInput")
    wvb_d = nc.dram_tensor("wvb", [E, NKV * D], bf16, kind="ExternalInput")
    wob_d = nc.dram_tensor("wob", [NH * D, E], bf16, kind="ExternalInput")
    cos_d = nc.dram_tensor("cosd", [P, T], f16, kind="ExternalInput")
    sin_d = nc.dram_tensor("sind", [P, T], f16, kind="ExternalInput")
    mk_d = nc.dram_tensor("mkd", [4, P, CH], bf16, kind="ExternalInput")
    o_d = nc.dram_tensor("od", [T, E], bf16, kind="ExternalOutput")

    xb_r = xb_d.rearrange("(eb p) t -> p eb t", p=P)
    wqb_r = wqb_d.rearrange("(eb p) m -> p eb m", p=P)
    wkb_r = wkb_d.rearrange("(eb p) m -> p eb m", p=P)
    wvb_r = wvb_d.rearrange("(eb p) m -> p eb m", p=P)
    wob_r = wob_d.rearrange("(h p) e -> p h e", p=P)
    mk_r = mk_d.rearrange("f p c -> p f c")
    o_r = o_d.rearrange("(tb p) e -> p tb e", p=P)

    with tile.TileContext(nc) as tc:
        with (
            tc.tile_pool(name="singles", bufs=1) as sg,
            tc.tile_pool(name="ropet", bufs=2) as rp,
            tc.tile_pool(name="expp", bufs=4) as ep_,
            tc.tile_pool(name="e8pp", bufs=2) as e8_,
            tc.tile_pool(name="normp", bufs=1) as np_,
            tc.tile_pool(name="outst", bufs=2) as op_,
        ):

            def emit_body():
                wkb_sb = sg.tile([P, EB, NKV * D], bf16, name="wkb_sb", tag="wkb_sb")
                xb_sb = sg.tile([P, EB, T], bf16, name="xb_sb", tag="xb_sb")
                wvb_sb = sg.tile([P, EB, NKV * D], bf16, name="wvb_sb", tag="wvb_sb")
                wqb_sb = sg.tile([P, EB, NH * D], bf16, name="wqb_sb", tag="wqb_sb")
                cos_sb = sg.tile([P, T], f16, name="cos_sb", tag="cos_sb")
                sin_sb = sg.tile([P, T], f16, name="sin_sb", tag="sin_sb")
                wob_sb = sg.tile([P, NH, E], bf16, name="wob_sb", tag="wob_sb")
                mk_sb = sg.tile([P, 4, CH], bf16, name="mk_sb", tag="mk_sb")

                # One DMA queue in strict consumption order: wk and x
                # interleaved per e-block (the K wave consumes block eb as
                # soon as wk[eb]+x[eb] land), rope tables mid-stream, then
                # wv/wq/mk/wo.
                nc.sync.dma_start(wkb_sb[:, 0:1], wkb_r[:, 0:1])
                # first e-block of x at chunk grain so the wave starts ASAP
                for tci in range(NTQ):
                    xsl = slice(CH * tci, CH * (tci + 1))
                    nc.sync.dma_start(xb_sb[:, 0, xsl], xb_r[:, 0, xsl])
                nc.sync.dma_start(wkb_sb[:, 1:2], wkb_r[:, 1:2])
                nc.sync.dma_start(xb_sb[:, 1:2], xb_r[:, 1:2])
                for e2 in range(1, EB // 2):
                    sl2 = slice(2 * e2, 2 * e2 + 2)
                    nc.sync.dma_start(wkb_sb[:, sl2], wkb_r[:, sl2])
                    nc.sync.dma_start(xb_sb[:, sl2], xb_r[:, sl2])
                    if e2 == 4:
                        nc.sync.dma_start(cos_sb[:], cos_d[:])
                        nc.sync.dma_start(sin_sb[:], sin_d[:])
                nc.sync.dma_start(wvb_sb[:], wvb_r[:])
                nc.sync.dma_start(wqb_sb[:], wqb_r[:])
                nc.sync.dma_start(mk_sb[:], mk_r[:])
                nc.sync.dma_start(wob_sb[:], wob_r[:])

                ones_sb = sg.tile([P, 1], bf16, name="ones_sb", tag="ones_sb")
                nc.vector.memset(ones_sb[:], 1.0)
                # fp8 ones for the DoubleRow denominator (Ko stride 16 bytes)
                ones8_sb = sg.tile([P, 2, 16], f8, name="ones8_sb", tag="ones8_sb")
                nc.vector.memset(ones8_sb[:], 1.0)

                kT_sb = [sg.tile([P, T], bf16, name=f"kT{g}", tag=f"kT{g}") for g in range(NKV)]
                qT_sb = [sg.tile([P, T], bf16, name=f"qT{h}", tag=f"qT{h}") for h in range(NH)]
                v_sb = sg.tile([P, NTB, NKV * D], bf16, name="v_sb", tag="v_sb")
                # fp8 v packed by t-block pairs for DoubleRow AV on full blocks
                v8h_sb = sg.tile([P, NTB // 2, 2, NKV * D], f8, name="v8h_sb", tag="v8h_sb")
                # attention outputs [d, h, t], consumed head-sliced by wo
                aob_sb = sg.tile([P, NH, T], bf16, name="aob_sb", tag="aob_sb")

                def rope_chunk(dest, sl, ps):
                    # dest[:, sl] = ps * cos + swap_halves(ps) * sin (sin rows 0:64 pre-negated)
                    t1 = rp.tile([P, CH], f32, name="ropet1", tag="ropet1")
                    nc.vector.tensor_mul(t1[:], ps[:], cos_sb[:, sl])
                    t2 = rp.tile([P, CH], f32, name="ropet2", tag="ropet2")
                    nc.vector.tensor_mul(t2[0:64, :], ps[64:128, :], sin_sb[0:64, sl])
                    nc.vector.tensor_mul(t2[64:128, :], ps[0:64, :], sin_sb[64:128, sl])
                    nc.vector.tensor_add(dest[:, sl], t1[:], t2[:])

                # ---- projections share one 8-slot PSUM ring: the K wave holds
                # all 8 slots e-block-outer (DMA-paced), then V tiles cycle
                # through slots as each K rope frees one (no pool barrier).
                with tc.tile_pool(name="pjw", bufs=8, space="PSUM") as pjw:
                    psk = [[pjw.tile([P, CH], f32, name=f"psk{g}_{t}", tag="pjw")
                            for t in range(NTQ)] for g in range(NKV)]
                    for eb in range(EB):
                        for g in range(NKV):
                            csl = slice(D * g, D * (g + 1))
                            for tci in range(NTQ):
                                xsl = slice(CH * tci, CH * (tci + 1))
                                nc.tensor.matmul(
                                    psk[g][tci][:],
                                    wkb_sb[:, eb, csl], xb_sb[:, eb, xsl],
                                    start=(eb == 0), stop=(eb == EB - 1))
                    for g in range(NKV):
                        for tci in range(NTQ):
                            rope_chunk(kT_sb[g], slice(CH * tci, CH * (tci + 1)),
                                       psk[g][tci])

                    # ---- V projection: chunk-major, e-block-inner
                    for u in range(8):
                        psv = pjw.tile([P, CH], f32, name="psv", tag="pjw")
                        for k2 in range(2):
                            tb = 2 * u + k2
                            tsl = slice(P * tb, P * (tb + 1))
                            for eb in range(EB):
                                nc.tensor.matmul(
                                    psv[:, 256 * k2:256 * (k2 + 1)],
                                    xb_sb[:, eb, tsl], wvb_sb[:, eb],
                                    start=(eb == 0), stop=(eb == EB - 1))
                        # ACT: idle during projections, DVE is busy with rope.
                        # psv holds t-blocks (2u, 2u+1) side by side = exactly
                        # v_sb[:, 2u:2u+2] and the pair-packed v8h slot for u.
                        nc.scalar.activation(v_sb[:, 2 * u:2 * u + 2, :], psv[:], Copy)
                        nc.scalar.activation(v8h_sb[:, u], psv[:], Copy)

                with (
                    tc.tile_pool(name="pj", bufs=2, space="PSUM") as pj,
                    tc.tile_pool(name="ps_s", bufs=3, space="PSUM") as ps_s,
                    tc.tile_pool(name="ps_o", bufs=2, space="PSUM") as ps_o,
                    tc.tile_pool(name="ps_m", bufs=1, space="PSUM") as ps_m,
                ):
                    # ---- Q projection: chunk-major per head, on the attention
                    # pj ring (the pool barrier above only waits on V's copies;
                    # attention then overlaps Q's DVE rope tail with no stall)
                    def q_chunk(h, tci):
                        csl = slice(D * h, D * (h + 1))
                        xsl = slice(CH * tci, CH * (tci + 1))
                        psq = pj.tile([P, CH], f32, name="psq", tag="pj")
                        for eb in range(EB):
                            nc.tensor.matmul(
                                psq[:],
                                wqb_sb[:, eb, csl], xb_sb[:, eb, xsl],
                                start=(eb == 0), stop=(eb == EB - 1))
                        rope_chunk(qT_sb[h], xsl, psq)

                    # half the Q chunks are deferred into the tci=0 attention
                    # stream below: those units are tiny (4 blocks), so PE
                    # stalls on exp latency with nothing to overlap — Q
                    # matmuls are exp-independent filler. Upfront: only the
                    # chunks attention needs before the deferred ones land.
                    for h, tci in [(0, 0), (0, 1), (0, 2), (0, 3),
                                   (1, 0), (1, 1), (1, 2), (2, 0)]:
                        q_chunk(h, tci)
                    q_deferred = [(3, 0), (2, 1), (3, 1), (2, 2),
                                  (3, 2), (2, 3), (3, 3), (1, 3)]

                    # one PSUM bank holds both s_row accumulators: consecutive
                    # units alternate base partition 0/32 so the next unit's
                    # denominator matmuls never wait on the previous copy-out
                    srow_bank = ps_m.tile([P, CH], f32, name="srow_bank", tag="srow_bank")
                    unit_idx = [0]

                    def attn(h, tci):
                        g = h // 2
                        sl = slice(CH * tci, CH * (tci + 1))
                        ntk = 4 * tci + 4
                        o_ps = ps_o.tile([P, CH], f32, name="o_ps", tag="o_ps")
                        p0 = 0  # DoubleRow matmul dst must start at partition 0
                        unit_idx[0] += 1
                        s_row = srow_bank[p0:p0 + 1, :]
                        # full blocks (j < 4*tci) in fp8 pairs: exp writes e4m3
                        # halves of a pair tile (logits are O(4), so exp stays
                        # far under e4m3's 240 max), then one DoubleRow AV and
                        # one DoubleRow denominator matmul per pair — half the
                        # PE streaming of the bf16 form per measured rates.
                        nfull = 4 * tci
                        for pi in range(nfull // 2):
                            e8p = e8_.tile([P, 2, CH], f8, name="e8p", tag="e8p")
                            for half in range(2):
                                j = 2 * pi + half
                                s_ps = ps_s.tile([P, CH], f32, name="s_ps", tag="s_ps")
                                nc.tensor.matmul(
                                    s_ps[:], kT_sb[g][:, P * j:P * (j + 1)], qT_sb[h][:, sl],
                                    start=True, stop=True,
                                )
                                nc.scalar.activation(e8p[:, half], s_ps[:], Exp, scale=SCALE)
                            nc.tensor.matmul(
                                o_ps[:], v8h_sb[:, pi, :, D * g:D * (g + 1)], e8p[:],
                                start=(pi == 0), stop=False, perf_mode=DR,
                            )
                            nc.tensor.matmul(
                                s_row[:], ones8_sb[:, :, 0:1], e8p[:],
                                start=(pi == 0), stop=False, perf_mode=DR,
                            )
                        # diagonal blocks, bf16: causal mask multiply needed
                        for j in range(nfull, ntk):
                            di = j - 4 * tci
                            c0 = P * di if di > 0 else 0
                            qsl = slice(CH * tci + c0, CH * (tci + 1))
                            s_ps = ps_s.tile([P, CH], f32, name="s_ps", tag="s_ps")
                            nc.tensor.matmul(
                                s_ps[:, c0:], kT_sb[g][:, P * j:P * (j + 1)], qT_sb[h][:, qsl],
                                start=True, stop=True,
                            )
                            e_t = ep_.tile([P, CH], bf16, name="e_t", tag="e_t")
                            nc.scalar.activation(e_t[:, c0:], s_ps[:, c0:], Exp, scale=SCALE)
                            nc.vector.tensor_mul(e_t[:, c0:], e_t[:, c0:], mk_sb[:, di, c0:])
                            nc.tensor.matmul(
                                o_ps[:, c0:], v_sb[:, j, D * g:D * (g + 1)], e_t[:, c0:],
                                start=(j == 0), stop=(j == ntk - 1),
                            )
                            nc.tensor.matmul(
                                s_row[:, c0:], ones_sb[:], e_t[:, c0:],
                                start=(j == 0), stop=(j == ntk - 1),
                            )
                        # normalize chain spread across Pool/DVE (DVE is the
                        # scarce engine during attention)
                        srow_sb = np_.tile([1, CH], f32, name="srow_sb", tag="srow_sb")
                        nc.vector.tensor_copy(out=srow_sb[:], in_=s_row[:])
                        rec = np_.tile([1, CH], f32, name="rec", tag="rec")
                        nc.vector.reciprocal(rec[:], srow_sb[:])
                        bc = np_.tile([P, CH], f32, name="bc", tag="bc")
                        nc.gpsimd.partition_broadcast(bc[:], rec[:])
                        nc.vector.tensor_mul(aob_sb[:, h, sl], o_ps[:], bc[:])

                    def wo_tb(tb, last=False):
                        tsl = slice(P * tb, P * (tb + 1))
                        ost = op_.tile([P, E], bf16, name="ost", tag="ost")
                        for n in range(4):
                            nsl = slice(CH * n, CH * (n + 1))
                            wop = pj.tile([P, CH], f32, name="wop", tag="pj")
                            for h in range(NH):
                                nc.tensor.matmul(
                                    wop[:],
                                    aob_sb[:, h, tsl],
                                    wob_sb[:, h, nsl],
                                    start=(h == 0), stop=(h == NH - 1),
                                )
                            # PSUM->SBUF copies split DVE/ACT (ACT is
                            # exp-heavy during attention, idle on last)
                            on_act = (n % 2 == 0) if last else (n == 3)
                            if on_act:
                                nc.scalar.activation(ost[:, nsl], wop[:], Copy)
                            else:
                                nc.vector.tensor_copy(out=ost[:, nsl], in_=wop[:])
                            if last:
                                # per-chunk DMA keeps the post-PE tail short
                                nc.sync.dma_start(o_r[:, tb, nsl], ost[:, nsl])
                        if not last:
                            nc.sync.dma_start(o_r[:, tb, :], ost[:])

                    # chunk-ascending attention; each chunk's wo spread across
                    # the next chunk's heads (one t-block after each head) so
                    # the softmax normalize chain has drained and the final
                    # chunk-boundary normalize wait is filled with wo work.
                    for tci in range(NTQ):
                        for h in range(NH):
                            if tci == 0:
                                # two deferred Q chunks before each unit;
                                # attn(3, 0) needs q(3, 0), emitted first
                                q_chunk(*q_deferred.pop(0))
                                q_chunk(*q_deferred.pop(0))
                            attn(h, tci)
                            if tci > 0:
                                wo_tb(4 * (tci - 1) + h)
                    for tb in range(4 * (NTQ - 1), 4 * NTQ):
                        wo_tb(tb, last=True)

            if reps > 1:
                with tc.For_i(0, reps, 1):
                    emit_body()
            else:
                emit_body()

    nc.finalize()
    return nc


def get_nc(reps=1):
    if reps not in _NC_CACHE:
        _NC_CACHE[reps] = _build_nc(reps)
    return _NC_CACHE[reps]


def make_host_inputs(x, wq, wk, wv, wo):
    """Returns per-core in_maps (list of 8 dicts)."""
    perm = np.concatenate([np.arange(0, D, 2), np.arange(1, D, 2)])
    wq4 = np.asarray(wq, np.float32).reshape(E, H, D)[:, :, perm]
    wk4 = np.asarray(wk, np.float32).reshape(E, KVH, D)[:, :, perm]
    wv4 = np.asarray(wv, np.float32).reshape(E, KVH, D)
    wo4 = np.asarray(wo, np.float32).reshape(H, D, E)
    xT = np.ascontiguousarray(np.transpose(np.asarray(x, np.float32), (0, 2, 1)))

    # rope tables (fp16; sin rows 0:64 pre-negated for the half-swap form)
    invf = 1.0 / (np.float32(THETA) ** (np.arange(0, D, 2, dtype=np.float32) / np.float32(D)))
    ang = np.arange(T, dtype=np.float32)[None, :] * invf[:, None]     # [64, T]
    cosv = np.cos(ang).astype(np.float32)
    sinv = np.sin(ang).astype(np.float32)
    cos_h = np.concatenate([cosv, cosv], 0).astype(F16)
    sin_h = np.concatenate([-sinv, sinv], 0).astype(F16)

    ii = np.arange(P)[:, None]
    jj = np.arange(CH)[None, :]
    mk_h = np.stack([(jj >= ii + P * di) for di in range(4)]).astype(BF16)

    in_maps = []
    for c in range(8):
        b, hg = divmod(c, 4)
        qs = slice(4 * hg, 4 * hg + 4)
        ks = slice(2 * hg, 2 * hg + 2)
        in_maps.append({
            "xb": xT[b].astype(BF16),
            "wqb": np.ascontiguousarray(wq4[:, qs].reshape(E, NH * D)).astype(BF16),
            "wkb": np.ascontiguousarray(wk4[:, ks].reshape(E, NKV * D)).astype(BF16),
            "wvb": np.ascontiguousarray(wv4[:, ks].reshape(E, NKV * D)).astype(BF16),
            "wob": np.ascontiguousarray(wo4[qs].reshape(NH * D, E)).astype(BF16),
            "cosd": cos_h,
            "sind": sin_h,
            "mkd": mk_h,
        })
    return in_maps


def gather_results(per_core_od):
    """Sum per-core partials."""
    out = np.zeros((B, T, E), np.float32)
    for c in range(8):
        out[c // 4] += per_core_od[c].astype(np.float32)
    return out


def kernel(x, mask, wq, wk, wv, wo, **extra):
    from concourse.bass_utils import run_bass_kernel_spmd

    nc = get_nc()
    in_maps = make_host_inputs(x, wq, wk, wv, wo)
    res = run_bass_kernel_spmd(nc, in_maps, core_ids=list(range(8)))
    return gather_results([res.results[c]["od"] for c in range(8)])
